# revision 5
# baseline (speedup 1.0000x reference)
"""ConcatCritic fused pair-grid MLP on 8 Trainium2 NeuronCores.

Math (reference):
    hx = x @ W1[:DX]                      # [B, H]
    hy = y @ W1[DX:] + b1                 # [B, H]
    h  = relu(hx[:,None,:] + hy[None,:,:])        # [B, B, H]
    h2 = relu(h @ W2 + b2)                        # [B, B, H]
    out[i, j] = (h2 @ W3)[i, j, 0] + b3           # [B, B]

Sharding: data-parallel over i (x rows). Each of the 8 cores computes a
[64, 512] slab of scores. y / W1 / W2 / W3 / b* are replicated.

Per-core dataflow (matmul operands fp16 — 1 cycle/row on the PE and
2-byte SBUF operands unlock the DVE 4x path; rel-err budget 2e-2 vs
~6e-4 measured for fp16):
    prep:  hyT[h, j]  = (W1y.T @ yT)  -> fp16     [256, 512]
           bias[h, i] = (W1x.T @ xT) + b1  (f32)  [256, 64]
    per i: A_iT[h, j]  = relu(hyT + bias[:, i])   fp16   (DVE, 4x)
           Z_iT[m, j]  = W2.T @ A_iT   (PSUM f32) (4 fp16 matmuls)
           Z2_iT[m, j] = relu(Z_iT + b2) -> fp16  (c0: ACT, c1: DVE)
           score[i, j] += W3.T @ Z2_iT            (2 fp16 matmuls)
    The 128 score matvecs all accumulate into ONE [64, 512] PSUM tile:
    the lhsT for row i is a shifted slice of a one-hot W3 "trick" region
    (column 64 holds W3, the rest zeros), so matvec i writes only
    partition i. One PSUM->SBUF copy + one DMA at the end. b3 is added
    on the host after the gather.

PE pipelining: the score matmuls for row i are emitted AFTER the Z
matmuls of row i+1, so the PE never stalls waiting for the Z2 relu of
the current row — Z2(i) is produced (ACT/DVE) while the PE streams
Z(i+1). This removes the per-iteration PE bubble of the serial order.

Sync-wait discipline: this walrus permits only ONE semaphore wait per
compute instruction. The f32 constants live in one [128, 1092] DRAM
tensor and the fp16 weights in one [128, 768] f16 tensor (HW scrambles
size-changing bitcasts, so fp16 data needs its own typed tensor), each
loaded by a single SWDGE DMA; each engine "pre-touches" the f32 pack
once so later instructions never need a separate DMA wait, and the f16
pack's only reader is the PE. Engine assignments per chunk are fixed (never
alternating) so cross-engine WAR releases are always covered by waits
already observed; _legalize_waits spills any excess into EventSemaphores.
"""

import numpy as np

B = 512
DX = 128
DY = 128
H = 256
P = 128          # partitions
HC = H // P      # h chunks (2)
NCORES = 8
BS = B // NCORES  # 64 rows of x per core

# packed-constant column offsets (fp32 words per partition)
OFF_YT = 0            # [512]     y.T (f32)
OFF_XT = 512          # [64]      x_shard.T (f32)
OFF_W1X = 576         # [256]     W1[:DX] (f32)
OFF_W1Y = 832         # [256]     W1[DX:] (f32)
OFF_B1 = 1088         # [2]       b1 chunks (read via bitcast->f32)
OFF_B2 = 1090         # [2]       b2 chunks (read via bitcast->f32)
PACK_COLS = 1092
# fp16 weight pack (separate f16 DRAM tensor -- HW scrambles size-changing
# bitcasts of the f32 pack, so fp16 data gets its own typed tensor/DMA):
OFFH_W2 = 0           # [2, 256]  W2 k-chunks: [p, c, m]
OFFH_W3T = 512        # [2, 128]  one-hot W3 trick: [p, c, col]
PACKH_COLS = 768

_cache = {}


def _build_nc(legalize=True, reps=1, loop_reps=0):
    import concourse.bass as bass
    import concourse.tile as tile
    import concourse.mybir as mybir

    f32 = mybir.dt.float32
    f32r = mybir.dt.float32r
    f16 = mybir.dt.float16
    Alu = mybir.AluOpType
    Act = mybir.ActivationFunctionType

    nc = bass.Bass(
        trn_type="TRN2",
        target_bir_lowering=False,
        debug=False,
        num_devices=NCORES,
    )

    d_pack = nc.dram_tensor("pack", [P, PACK_COLS], f32r, kind="ExternalInput")
    d_packh = nc.dram_tensor("packh", [P, PACKH_COLS], f16, kind="ExternalInput")
    d_out = nc.dram_tensor("out", [BS, B], f32, kind="ExternalOutput")

    with tile.TileContext(nc) as tc:
        with (
            tc.tile_pool(name="singles", bufs=1) as singles,
            tc.tile_pool(name="apool0", bufs=4) as apool0,
            tc.tile_pool(name="apool1", bufs=4) as apool1,
            tc.tile_pool(name="z2pool0", bufs=4) as z2pool0,
            tc.tile_pool(name="z2pool1", bufs=4) as z2pool1,
            tc.tile_pool(name="zpool", bufs=3, space="PSUM") as zpool,
            tc.tile_pool(name="spool", bufs=1, space="PSUM") as spool,
        ):
            pk = singles.tile([P, PACK_COLS], f32r)
            nc.gpsimd.dma_start(pk[:], d_pack[:])
            pkh = singles.tile([P, PACKH_COLS], f16)
            nc.gpsimd.dma_start(pkh[:], d_packh[:])

            def b1_col(c):
                return pk[:, OFF_B1 + c: OFF_B1 + c + 1].bitcast(f32)

            def b2_col(c):
                return pk[:, OFF_B2 + c: OFF_B2 + c + 1].bitcast(f32)

            # one op per engine touching the packed tile: advances each
            # engine's observed DMA clock so no later op needs a DMA wait
            scratch = singles.tile([P, 3], f32)
            nc.vector.tensor_copy(scratch[:, 0:1], b1_col(0))
            nc.scalar.copy(scratch[:, 1:2], b2_col(0))
            nc.gpsimd.tensor_copy(scratch[:, 2:3], b1_col(1))

            def w2_lhsT(c, m):
                o = OFFH_W2 + c * H + m * P
                return pkh[:, o: o + P]

            def w3_lhsT(c, i):
                o = OFFH_W3T + c * P + P // 2 - i
                return pkh[:, o: o + BS]

            sb_hy = singles.tile([P, HC, B], f16)
            sb_bias = singles.tile([P, HC, BS], f32)
            outbuf = singles.tile([BS, B], f32)

            # ---- prep: hyT (fp16) and per-row bias (f32) ----
            for c in range(HC):
                ps_hy = zpool.tile([P, HC, B], f32, tag="z")
                nc.tensor.matmul(
                    ps_hy[:, 0, :],
                    pk[:, OFF_W1Y + c * P: OFF_W1Y + (c + 1) * P],
                    pk[:, OFF_YT: OFF_YT + B],
                    start=True,
                    stop=True,
                )
                nc.vector.tensor_copy(sb_hy[:, c, :], ps_hy[:, 0, :])

                ps_hx = zpool.tile([P, HC, B], f32, tag="z")
                nc.tensor.matmul(
                    ps_hx[:, 0, :BS],
                    pk[:, OFF_W1X + c * P: OFF_W1X + (c + 1) * P],
                    pk[:, OFF_XT: OFF_XT + BS],
                    start=True,
                    stop=True,
                )
                nc.vector.tensor_scalar(
                    sb_bias[:, c, :], ps_hx[:, 0, :BS], b1_col(c), None, Alu.add
                )

            score_ps = spool.tile([BS, B], f32)

            # ---- main loop over the 64 x-rows of this core ----
            # (reps>1 replicates the whole loop for slope-based device
            # timing; each rep restarts the score accumulation;
            # loop_reps>0 wraps the body in a device-side For_i instead)
            import contextlib
            loop_cm = (
                tc.For_i(0, loop_reps, 1) if loop_reps
                else contextlib.nullcontext()
            )
            def perm(idx):
                return (idx % 2) * (BS // 2) + idx // 2

            with loop_cm:
              for rep in range(reps):
                def emit_a(i):
                    a0 = apool0.tile([P, B], f16, tag="a0")
                    a1 = apool1.tile([P, B], f16, tag="a1")
                    nc.vector.tensor_scalar(
                        a0[:], sb_hy[:, 0, :], sb_bias[:, 0, i:i + 1],
                        0.0, Alu.add, Alu.max,
                    )
                    nc.vector.tensor_scalar(
                        a1[:], sb_hy[:, 1, :], sb_bias[:, 1, i:i + 1],
                        0.0, Alu.add, Alu.max,
                    )
                    return [a0, a1]

                n_score = [0]

                def emit_score(pi, pz2, last=False):
                    for c in range(HC):
                        nc.tensor.matmul(
                            score_ps[:],
                            w3_lhsT(c, pi),
                            pz2[c][:],
                            start=(n_score[0] == 0 and c == 0),
                            stop=(last and c == HC - 1),
                            skip_group_check=True,
                        )
                    n_score[0] += 1

                n_score[0] = 0
                a_next = emit_a(perm(0))   # prologue: a for idx 0
                hist = []                  # pending (i, z2) score rows
                for idx in range(BS):
                  i = perm(idx)
                  a = a_next
                  if idx + 1 < BS:
                      a_next = emit_a(perm(idx + 1))

                  z = zpool.tile([P, HC, B], f32, tag="z")
                  for m in range(HC):
                      for c in range(HC):
                          nc.tensor.matmul(
                              z[:, m, :],
                              w2_lhsT(c, m),
                              a[c][:],
                              start=(c == 0),
                              stop=(c == HC - 1),
                              skip_group_check=True,
                          )

                  # score for row idx-2: its z2 is two iterations old, so
                  # the PE wait is pre-satisfied (no per-iter sem latency)
                  if len(hist) == 2:
                      emit_score(*hist.pop(0))

                  z20 = z2pool0.tile([P, B], f16, tag="z20")
                  z21 = z2pool1.tile([P, B], f16, tag="z21")
                  nc.scalar.activation(
                      z20[:], z[:, 0, :], Act.Relu, bias=b2_col(0), scale=1.0
                  )
                  nc.vector.tensor_scalar(
                      z21[:], z[:, 1, :], b2_col(1), 0.0, Alu.add, Alu.max,
                  )
                  hist.append((i, [z20, z21]))

                # drain the last two pending score rows
                for k, (pi, pz2) in enumerate(hist):
                    emit_score(pi, pz2, last=(k == len(hist) - 1))
                hist.clear()

            nc.vector.tensor_copy(outbuf[:], score_ps[:])
            nc.sync.dma_start(d_out[:], outbuf[:])

    if legalize:
        _legalize_waits(nc)
    return nc


def _legalize_waits(nc):
    """walrus accepts only ONE sync wait per compute instruction (and two
    per EventSemaphore). Tile sometimes leaves 2-3 waits on an instruction;
    spill the excess into EventSemaphore instructions inserted just before
    it on the same engine queue."""
    import concourse.mybir as mybir

    n_spilled = 0
    for f in nc.m.functions:
        for bb in f.blocks:
            insts = bb.instructions
            i = 0
            while i < len(insts):
                inst = insts[i]
                si = inst.sync_info
                if (
                    si is None
                    or not si.on_wait
                    or len(si.on_wait) <= 1
                    or inst.opcode == "EventSemaphore"
                ):
                    i += 1
                    continue
                waits = list(si.on_wait)
                keep, spill = waits[-1], waits[:-1]
                k = 0
                while spill:
                    chunk, spill = spill[:2], spill[2:]
                    ev = mybir.InstEventSemaphore(
                        name=f"{inst.name}-lw{k}", ins=[], outs=[]
                    )
                    ev.engine = inst.engine
                    ev.sync_info = mybir.SyncInfo(on_wait=chunk, on_update=[])
                    insts.insert(i, ev)
                    i += 1
                    k += 1
                    n_spilled += 1
                inst.sync_info = mybir.SyncInfo(
                    on_wait=[keep], on_update=list(si.on_update or [])
                )
                i += 1
    return n_spilled


def prep_inputs(x, y, W1, b1, W2, b2, W3):
    """Host-side sharding/layout. Returns per-core input maps."""
    x = np.ascontiguousarray(np.asarray(x, dtype=np.float32))
    y = np.ascontiguousarray(np.asarray(y, dtype=np.float32))
    W1 = np.asarray(W1, dtype=np.float32)
    b1 = np.asarray(b1, dtype=np.float32)
    W2 = np.asarray(W2, dtype=np.float32)
    b2 = np.asarray(b2, dtype=np.float32)
    W3 = np.asarray(W3, dtype=np.float32)

    pack = np.zeros((P, PACK_COLS), dtype=np.float32)
    packh = np.zeros((P, PACKH_COLS), dtype=np.float16)
    # W2 k-chunks as fp16: packh[p, c*H + m] = W2[c*P + p, m]
    packh[:, OFFH_W2:OFFH_W2 + HC * H] = (
        np.transpose(W2.reshape(HC, P, H), (1, 0, 2)).reshape(P, HC * H)
        .astype(np.float16)
    )
    # one-hot W3 trick as fp16: column 64 of each [128, 128] region
    for c in range(HC):
        packh[:, OFFH_W3T + c * P + P // 2] = (
            W3[c * P:(c + 1) * P, 0].astype(np.float16)
        )
    pack[:, OFF_YT:OFF_YT + B] = y.T
    pack[:, OFF_W1X:OFF_W1X + H] = W1[:DX]
    pack[:, OFF_W1Y:OFF_W1Y + H] = W1[DX:]
    pack[:, OFF_B1:OFF_B1 + HC] = b1.reshape(HC, P).T
    pack[:, OFF_B2:OFF_B2 + HC] = b2.reshape(HC, P).T

    in_maps = []
    for core in range(NCORES):
        pc = pack.copy()
        pc[:, OFF_XT:OFF_XT + BS] = x[core * BS:(core + 1) * BS].T
        in_maps.append({"pack": pc, "packh": packh.copy()})
    return in_maps


def kernel(x, y, W1, b1, W2, b2, W3, b3):
    from concourse.bass_utils import run_bass_kernel_spmd

    if "nc" not in _cache:
        _cache["nc"] = _build_nc()
    nc = _cache["nc"]

    in_maps = prep_inputs(x, y, W1, b1, W2, b2, W3)
    res = run_bass_kernel_spmd(nc, in_maps, core_ids=list(range(NCORES)))
    out = np.concatenate([res.results[c]["out"] for c in range(NCORES)], axis=0)
    out = out + np.float32(np.asarray(b3, dtype=np.float32).reshape(()))
    return out.astype(np.float32)
